# revision 4
# baseline (speedup 1.0000x reference)
"""LSTM autoencoder (8 stacked LSTM layers, B=128 T=512 H=256) on 8 trn2 cores.

Strategy: pipeline the 8 layers across the 8 cores (layer l on core l), full
batch B=128 per core. Everything on-chip lives transposed: [units -> 128
partitions, batch -> free]. Matmuls are bf16 (weights stationary), PSUM/state
fp32. Chunks of C timesteps flow core->core via an AllGather ring with a skew
of 2 rounds so the collective hides behind compute. Core 0 reads its input
(host-pre-transposed x) from section 8 of the gather buffer, filled locally
each round; cores 1..7 read section (core-1) via a dynamic-slice DMA whose
index comes from a per-core input tensor.
"""
import sys, os, time
sys.path.insert(0, "/opt/trn_rl_repo")
import numpy as np
import ml_dtypes

import concourse.bass as bass
import concourse.bacc as bacc
import concourse.mybir as mybir
from concourse import tile
from concourse.bass_utils import run_bass_kernel_spmd

BF16 = ml_dtypes.bfloat16
B, T, H = 128, 512, 256
NCORES, NLAYERS = 8, 8
C = 8                      # timesteps per chunk
NCHUNK = T // C
SKEW = 2                   # rounds between produce and consume
ROUNDS = NCHUNK + SKEW * (NCORES - 1)
NBUF = 2 * SKEW            # rotating gather buffers

F32 = mybir.dt.float32
BF = mybir.dt.bfloat16
AF = mybir.ActivationFunctionType

_cache = {}


def _build():
    nc = bacc.Bacc("TRN2", target_bir_lowering=False, debug=False,
                   num_devices=NCORES)
    wt = nc.dram_tensor("wt", [4, 8, 128, 128], BF, kind="ExternalInput")
    bias = nc.dram_tensor("bias", [8, 128], F32, kind="ExternalInput")
    xt = nc.dram_tensor("xt", [T, 128, 2, 128], BF, kind="ExternalInput")
    secv = nc.dram_tensor("secv", [1, 1], mybir.dt.int32, kind="ExternalInput")
    maskt = nc.dram_tensor("maskt", [ROUNDS, 128], F32, kind="ExternalInput")
    out = nc.dram_tensor("out", [T, 128, 2, 128], BF, kind="ExternalOutput")

    with tile.TileContext(nc) as tc:
        with (
            tc.tile_pool(name="const", bufs=1) as constp,
            tc.tile_pool(name="sb", bufs=3) as sb,
            tc.tile_pool(name="state", bufs=2) as statep,
            tc.tile_pool(name="ps", bufs=2, space="PSUM") as psp,
            tc.tile_pool(name="dram", bufs=1, space="DRAM") as dram,
        ):
            snd = [dram.tile([C, 128, 2, 128], BF, tag=f"snd{i}", name=f"snd{i}")
                   for i in range(2)]
            ag = [dram.tile([9 * C, 128, 2, 128], BF, tag=f"ag{i}", name=f"ag{i}")
                  for i in range(NBUF)]
            trash = dram.tile([C, 128, 2, 128], BF)

            # weights: [k, m, p, c] -> sbuf [p, (k m c)]
            wsb = constp.tile([128, 4 * 8 * 128], BF)
            for k in range(4):
                for m in range(8):
                    i = k * 8 + m
                    nc.sync.dma_start(wsb[:, i * 128:(i + 1) * 128], wt[k, m])
            bsb = constp.tile([128, 8], F32)
            nc.sync.dma_start(bsb, bias[:, :].rearrange("m p -> p m"))
            zer = constp.tile([128, 1024], BF)
            nc.vector.memset(zer, 0.0)

            def wtile(k, m):
                i = k * 8 + m
                return wsb[:, i * 128:(i + 1) * 128]

            # zero the two gather buffers that are read before ever AG-written,
            # and pre-fill their x sections with chunks 0 and 1
            for i in (NBUF - 2, NBUF - 1):
                for j in range(8 * C):
                    nc.sync.dma_start(
                        ag[i][j].rearrange("p k b -> p (k b)"), zer[:, :256])
            for pre in range(SKEW):
                nc.sync.dma_start(ag[(pre - SKEW) % NBUF][8 * C:9 * C],
                                  xt[pre * C:(pre + 1) * C])

            # per-core previous-section index
            secs = constp.tile([1, 1], mybir.dt.int32)
            nc.sync.dma_start(secs, secv[:, :])
            reg = nc.alloc_registers()
            nc.regs_load(reg, secs[0:1, 0:1])
            sec = nc.snap(reg, donate=True, min_val=0, max_val=8)

            h = statep.tile([128, 256], BF, tag="h")
            c = statep.tile([128, 256], F32, tag="c")
            nc.vector.memset(h, 0.0)
            nc.vector.memset(c, 0.0)

            PAIRS = ((4, 5, "g"), (2, 3, "f"), (0, 1, "i"), (6, 7, "o"))

            for r in range(ROUNDS):
                rdb = ag[(r - SKEW) % NBUF]
                wrb = ag[r % NBUF]
                sndb = snd[r % 2]

                # state reset mask for pipeline warmup
                mt = sb.tile([128, 1], F32, tag="mask")
                nc.sync.dma_start(mt, maskt[r:r + 1, :].rearrange("r p -> p r"))
                hm = statep.tile([128, 256], BF, tag="h")
                cm = statep.tile([128, 256], F32, tag="c")
                nc.vector.tensor_scalar_mul(hm, h, mt[:, 0:1])
                nc.vector.tensor_scalar_mul(cm, c, mt[:, 0:1])
                h, c = hm, cm

                rdblk = rdb[bass.ts(sec, C)]
                for s in range(C):
                    xin = sb.tile([128, 256], BF, tag="xin")
                    nc.sync.dma_start(
                        xin, rdblk[s].rearrange("p k b -> p (k b)"))

                    ps = {}
                    act = {}
                    for m0, m1, gn in PAIRS:
                        pp = psp.tile([128, 256], F32, tag=f"ps{gn}")
                        ps[gn] = pp
                        for half, m in ((0, m0), (1, m1)):
                            dst = pp[:, half * 128:(half + 1) * 128]
                            for k in range(4):
                                rhs = (xin[:, k * 128:(k + 1) * 128] if k < 2
                                       else h[:, (k - 2) * 128:(k - 1) * 128])
                                nc.tensor.matmul(dst, wtile(k, m), rhs,
                                                 start=(k == 0), stop=(k == 3))
                        gt = sb.tile([128, 256], BF, tag=f"a{gn}")
                        act[gn] = gt
                        fn = AF.Tanh if gn == "g" else AF.Sigmoid
                        for half, m in ((0, m0), (1, m1)):
                            nc.scalar.activation(
                                gt[:, half * 128:(half + 1) * 128],
                                pp[:, half * 128:(half + 1) * 128],
                                fn, bias=bsb[:, m:m + 1])

                    fc = sb.tile([128, 256], F32, tag="fc")
                    nc.vector.tensor_mul(fc, act["f"], c)
                    ig = sb.tile([128, 256], BF, tag="ig")
                    nc.vector.tensor_mul(ig, act["i"], act["g"])
                    cn = statep.tile([128, 256], F32, tag="c")
                    nc.vector.tensor_add(cn, fc, ig)
                    tch = sb.tile([128, 256], BF, tag="tch")
                    nc.scalar.activation(tch, cn, AF.Tanh)
                    hn = statep.tile([128, 256], BF, tag="h")
                    nc.vector.tensor_mul(hn, act["o"], tch)
                    h, c = hn, cn

                    nc.sync.dma_start(
                        sndb[s].rearrange("p k b -> p (k b)"), h)
                    od = (out[(r - SKEW * (NCORES - 1)) * C + s]
                          if r >= SKEW * (NCORES - 1) else trash[s])
                    nc.sync.dma_start(od.rearrange("p k b -> p (k b)"), h)

                # ship this round's chunk; refill the x section for round r+SKEW
                if r + SKEW < NCHUNK:
                    nc.sync.dma_start(wrb[8 * C:9 * C],
                                      xt[(r + SKEW) * C:(r + SKEW + 1) * C])
                nc.gpsimd.collective_compute(
                    "AllGather", mybir.AluOpType.bypass,
                    ins=[sndb[:, :, :, :].opt()],
                    outs=[wrb[0:8 * C].opt()],
                    replica_groups=[list(range(NCORES))],
                )
    nc.compile()
    return nc


def _prep_inputs(x, enc_params, dec_params):
    x = np.asarray(x, dtype=np.float32)
    layers = [(np.asarray(a), np.asarray(b), np.asarray(c_), np.asarray(d))
              for (a, b, c_, d) in list(enc_params) + list(dec_params)]
    xr = x.reshape(B, T, 2, 128)
    xT = np.ascontiguousarray(np.transpose(xr, (1, 3, 2, 0))).astype(BF16)
    in_maps = []
    for core in range(NCORES):
        Wih, Whh, bih, bhh = layers[core]
        W = np.concatenate([Wih, Whh], axis=1)          # [1024, 512]
        wt = np.zeros((4, 8, 128, 128), dtype=BF16)
        for k in range(4):
            for m in range(8):
                wt[k, m] = W[m * 128:(m + 1) * 128,
                             k * 128:(k + 1) * 128].T.astype(BF16)
        bia = (bih + bhh).astype(np.float32).reshape(8, 128)
        mask = np.ones((ROUNDS, 128), dtype=np.float32)
        mask[:SKEW * core + 1] = 0.0
        sec = np.array([[8 if core == 0 else core - 1]], dtype=np.int32)
        in_maps.append({"wt": wt, "bias": bia, "xt": xT, "secv": sec,
                        "maskt": mask})
    return in_maps


def kernel(x, enc_params, dec_params):
    if "nc" not in _cache:
        _cache["nc"] = _build()
    nc = _cache["nc"]
    in_maps = _prep_inputs(x, enc_params, dec_params)
    res = run_bass_kernel_spmd(nc, in_maps, core_ids=list(range(NCORES)))
    _cache["last"] = res
    o = np.asarray(res.results[NCORES - 1]["out"]).astype(np.float32)
    return np.ascontiguousarray(
        np.transpose(o, (3, 0, 2, 1)).reshape(B, T, 256))
